# revision 4
# baseline (speedup 1.0000x reference)
"""Trainium2 Bass kernel for nn_Critic — v2: single-copy input (no zr).

Differences vs v1 baseline:
  - input is shipped ONCE (feature-major zt only); the batch-major copy
    (zr) and its 17MB/core of DMA are gone.
  - L1 LayerNorm stats come from the zt chunks themselves: s1 = -mu via
    (-1/D)-ones matmuls over the 17 chunks, s2 = E[z^2] via (1/D)-ones
    matmuls over squared chunks (squares split between DVE and ACT to
    balance engine load).  var = s2 - mu^2 (f-major), quake-rsqrt nr=1.
  - no PE transposes / identity matrix / batch-major quake needed at all.
  - optional contiguous host layout (ZT_CONTIG): z pre-reordered so each
    tile's 16 main chunks are ONE [128, 8192] contiguous DMA.
Everything else (folded LN weights, aug -mu row, invs evac on DVE, tanh
+bias on ACT, PE ones-matmul L2/L3 stats, software pipelining) as v1.
"""

import os
import sys
import numpy as np

for _p in ("/opt/trn_rl_repo",):
    if os.path.isdir(_p) and _p not in sys.path:
        sys.path.append(_p)

from contextlib import ExitStack

import concourse.bass as bass  # noqa: E402
import concourse.tile as tile  # noqa: E402
from concourse import bacc, mybir  # noqa: E402
from concourse.bass_utils import run_bass_kernel_spmd  # noqa: E402
from concourse.tile_rust import add_dep_helper  # noqa: E402

NCORES = 8
B = 32768
BC = B // NCORES
INPUT_DIM = 2048
HALF = INPUT_DIM // 2
N_ACTIONS = 32
D = INPUT_DIM + N_ACTIONS  # 2080
H = 512
NT = 512
EPS = 1e-5
X_NORM = 50.0
V_NORM = 10.0

F16 = mybir.dt.float16
F32 = mybir.dt.float32
I32 = mybir.dt.int32
AF = mybir.ActivationFunctionType
OP = mybir.AluOpType

K1 = 17
K1_LAST = D - 16 * 128  # 32
RSQRT_MAGIC = 0x5F3759DF

ZT_CONTIG = True


def build_nc(bout: float, bc: int = BC, dmaq: str = None):
    if dmaq is None:
        dmaq = "scalar4"
    ntiles = bc // NT
    assert ntiles * NT == bc

    nc = bacc.Bacc("TRN2", target_bir_lowering=False, debug=False,
                   num_devices=NCORES)

    if ZT_CONTIG:
        ztc_d = nc.dram_tensor("ztc", [128, ntiles * 16 * NT], F16,
                               kind="ExternalInput").ap()
        zt16_d = nc.dram_tensor("zt16r", [K1_LAST, bc], F16,
                                kind="ExternalInput").ap()
    else:
        ztc_d = nc.dram_tensor("zt", [D, bc], F16, kind="ExternalInput").ap()
        zt16_d = None
    w1_d = nc.dram_tensor("w1a", [D + 1, H], F16, kind="ExternalInput").ap()
    w2_d = nc.dram_tensor("w2a", [H + 1, H], F16, kind="ExternalInput").ap()
    w3_d = nc.dram_tensor("w3a", [H + 1, H], F16, kind="ExternalInput").ap()
    cb_d = nc.dram_tensor("cb", [128, 12], F32, kind="ExternalInput").ap()
    wo_d = nc.dram_tensor("wout", [H, 1], F16, kind="ExternalInput").ap()
    q_d = nc.dram_tensor("q", [1, bc], F32, kind="ExternalOutput").ap()

    with tile.TileContext(nc) as tc:
        _emit(tc, ntiles, bout, ztc_d, zt16_d, w1_d, w2_d, w3_d, cb_d, wo_d,
              q_d, dmaq)

    nc.compile()
    return nc


def _emit(tc, ntiles, bout, ztc_d, zt16_d, w1_d, w2_d, w3_d, cb_d, wo_d, q_d,
          dmaq="scalar"):
    nc = tc.nc
    with ExitStack() as ctx:
        wp = ctx.enter_context(tc.tile_pool(name="wp", bufs=1))
        zt_p = ctx.enter_context(tc.tile_pool(name="ztp", bufs=3))
        h_p = ctx.enter_context(tc.tile_pool(name="hp", bufs=2))
        u_p = ctx.enter_context(tc.tile_pool(name="up", bufs=4))
        sq_p = ctx.enter_context(tc.tile_pool(name="sqp", bufs=20))
        sqb_p = ctx.enter_context(tc.tile_pool(name="sqbp", bufs=4))
        bc_p = ctx.enter_context(tc.tile_pool(name="bcp", bufs=3))
        st_p = ctx.enter_context(tc.tile_pool(name="stp", bufs=3))
        ps_y = ctx.enter_context(tc.tile_pool(name="psy", bufs=4, space="PSUM"))
        # one bank per stats pair: s1 at partition row 0, s2 at row 32
        ps_s = ctx.enter_context(tc.tile_pool(name="pss", bufs=3, space="PSUM"))
        ps_t = ctx.enter_context(tc.tile_pool(name="pst", bufs=1, space="PSUM"))

        # ---- persistent constants / weights ----
        w1main = wp.tile([128, 16, H], F16, tag="w1main")
        w1last = wp.tile([K1_LAST + 1, H], F16, tag="w1last")
        w1 = [w1main[:, k, :] for k in range(16)] + [w1last]
        w2main = wp.tile([128, 4, H], F16, tag="w2main")
        w2aug = wp.tile([1, H], F16, tag="w2aug")
        w2 = [w2main[:, k, :] for k in range(4)] + [w2aug]
        w3main = wp.tile([128, 4, H], F16, tag="w3main")
        w3aug = wp.tile([1, H], F16, tag="w3aug")
        w3 = [w3main[:, k, :] for k in range(4)] + [w3aug]
        cbT = wp.tile([128, 12], F32, tag="cbT")
        wo = wp.tile([128, 4, 1], F16, tag="wo")

        def load_weights():
            for hlf in range(2):
                nc.sync.dma_start(
                    out=w1main[:, hlf * 8:(hlf + 1) * 8, :],
                    in_=w1_d[hlf * 1024:(hlf + 1) * 1024, :]
                        .rearrange("(k p) h -> p k h", k=8))
            nc.sync.dma_start(out=w1last[:, :],
                              in_=w1_d[2048:2048 + K1_LAST + 1, :])
            wq = nc.sync if dmaq in ("scalar", "scalar4") else nc.scalar
            for wt, wa, wd in ((w2main, w2aug, w2_d), (w3main, w3aug, w3_d)):
                wq.dma_start(
                    out=wt[:, :, :],
                    in_=wd[0:H, :].rearrange("(k p) h -> p k h", k=4))
                wq.dma_start(out=wa[:, :], in_=wd[H:H + 1, :])
            wq.dma_start(out=cbT[:, :], in_=cb_d[:, :])
            wq.dma_start(
                out=wo[:, :, :],
                in_=wo_d[:, :].rearrange("(k p) o -> p k o", k=4))

        onesn = wp.tile([128, 1], F16, tag="onesn")  # -1/H (L2/L3 s1)
        nc.vector.memset(onesn[:, :], -1.0 / H)
        onesp = wp.tile([128, 1], F16, tag="onesp")  # +1/H (L2/L3 s2)
        nc.vector.memset(onesp[:, :], 1.0 / H)
        onesDn = wp.tile([128, 1], F16, tag="onesDn")  # -1/D (L1 s1)
        nc.vector.memset(onesDn[:, :], -1.0 / D)
        onesDp = wp.tile([128, 1], F16, tag="onesDp")  # +1/D (L1 s2)
        nc.vector.memset(onesDp[:, :], 1.0 / D)
        boutT = wp.tile([1, 1], F32, tag="boutT")
        nc.vector.memset(boutT[:, :], bout)
        qrow = wp.tile([1, ntiles * NT], F32, tag="qrow")

        def evac(py, bctile, htile, cbcol):
            """h = tanh(invs (.) psum + c).

            Split into [PSUM->SBUF f16 copy] + [f16 x f16 multiply] so the
            PSUM bank is released as soon as the matmul group finishes --
            the multiply (which waits on the invs broadcast, i.e. the whole
            stats/rsqrt chain) no longer gates PE's psum rotation."""
            u1 = u_p.tile([128, NT], F16, tag="u1")
            nc.vector.tensor_copy(u1[:, :], py[:, :])
            u = u_p.tile([128, NT], F16, tag="u")
            nc.vector.tensor_mul(u[:, :], u1[:, :], bctile[:, :])
            nc.scalar.activation(htile[:, :], u[:, :], AF.Tanh, bias=cbcol)

        def bcast(row_ap):
            r16 = bc_p.tile([1, NT], F16, tag="bc16", name="bc16")
            nc.vector.tensor_copy(r16[0:1, :], row_ap)
            t = bc_p.tile([128, NT], F16, tag="bc")
            inst = nc.gpsimd.partition_broadcast(t[:, :], r16[0:1, :])
            return t, inst

        def quake_rsqrt(v_ap, out_ap, tag, rows=1, nr=1):
            """out = 1/sqrt(v) elementwise; v, out: [rows, w] f32 SBUF."""
            w = v_ap.shape[-1]
            it = st_p.tile([rows, w], I32, tag=f"qi_{tag}", name=f"qi_{tag}")
            tt = st_p.tile([rows, w], F32, tag=f"qt_{tag}", name=f"qt_{tag}")
            nc.vector.tensor_scalar(out=it[:, :], in0=v_ap.bitcast(I32),
                                    scalar1=1, scalar2=None,
                                    op0=OP.arith_shift_right)
            nc.vector.tensor_scalar(out=it[:, :], in0=it[:, :],
                                    scalar1=-1, scalar2=RSQRT_MAGIC,
                                    op0=OP.mult, op1=OP.add)
            y = it[:, :].bitcast(F32)
            for r in range(nr):
                nc.vector.tensor_mul(tt[:, :], y, y)
                nc.vector.tensor_mul(tt[:, :], tt[:, :], v_ap)
                nc.vector.tensor_scalar(out=tt[:, :], in0=tt[:, :],
                                        scalar1=-0.5, scalar2=1.5,
                                        op0=OP.mult, op1=OP.add)
                nc.vector.tensor_mul(out_ap, y, tt[:, :])
                y = out_ap

        # ---------- per-tile emission pieces ----------

        def front_dma(it):
            """Allocate tile state + start input DMAs."""
            bs = it * NT
            fr = {"bs": bs}
            fr["zt16"] = zt_p.tile([K1_LAST + 1, NT], F16, tag="zt16",
                                   name="zt16")
            fr["invs1"] = st_p.tile([1, NT], F32, tag="invs", name="invs")
            ztmain = zt_p.tile([128, 16, NT], F16, tag="ztmain")
            if ZT_CONTIG:
                if dmaq == "split":
                    nc.sync.dma_start(
                        out=ztmain[:, 0:8, :],
                        in_=ztc_d[:, it * 16 * NT:it * 16 * NT + 8 * NT]
                            .rearrange("p (k n) -> p k n", k=8))
                    nc.scalar.dma_start(
                        out=ztmain[:, 8:16, :],
                        in_=ztc_d[:, it * 16 * NT + 8 * NT:(it + 1) * 16 * NT]
                            .rearrange("p (k n) -> p k n", k=8))
                elif dmaq == "scalar4":
                    for pc in range(4):
                        nc.scalar.dma_start(
                            out=ztmain[:, pc * 4:(pc + 1) * 4, :],
                            in_=ztc_d[:, it * 16 * NT + pc * 4 * NT:
                                      it * 16 * NT + (pc + 1) * 4 * NT]
                                .rearrange("p (k n) -> p k n", k=4))
                else:
                    q = nc.sync if dmaq == "sync" else nc.scalar
                    q.dma_start(
                        out=ztmain[:, :, :],
                        in_=ztc_d[:, it * 16 * NT:(it + 1) * 16 * NT]
                            .rearrange("p (k n) -> p k n", k=16))
                nc.sync.dma_start(out=fr["zt16"][0:K1_LAST, :],
                                  in_=zt16_d[:, bs:bs + NT])
            else:
                for hlf in range(2):
                    nc.scalar.dma_start(
                        out=ztmain[:, hlf * 8:(hlf + 1) * 8, :],
                        in_=ztc_d[hlf * 1024:(hlf + 1) * 1024, bs:bs + NT]
                            .rearrange("(k p) n -> p k n", k=8))
                nc.sync.dma_start(out=fr["zt16"][0:K1_LAST, :],
                                  in_=ztc_d[2048:2048 + K1_LAST, bs:bs + NT])
            fr["zts"] = [ztmain[:, k, :] for k in range(16)] + [fr["zt16"]]
            return fr

        def front_sq(it, fr, ks):
            """Square chunks ks; alternate DVE/ACT to balance load."""
            if "sqs" not in fr:
                fr["sqs"] = {}
            for k in ks:
                src = fr["zts"][k]
                rows = 128 if k < 16 else K1_LAST
                sq = sq_p.tile([128, NT], F16, tag="sq1", name="sq1")
                if k % 2 == 0:
                    nc.vector.tensor_mul(sq[0:rows, :], src[0:rows, :],
                                         src[0:rows, :])
                else:
                    nc.scalar.activation(sq[0:rows, :], src[0:rows, :],
                                         AF.Square)
                fr["sqs"][k] = sq

        def front_stat_mm(it, fr):
            """s1 = -mu, s2 = E[z^2] on PE; then var, rsqrt, aug row."""
            sst = ps_s.tile([128, NT], F32, tag="sst", name="sst")
            s1 = sst[0:1, :]
            s2 = sst[32:33, :]
            for k in range(K1):
                rows = 128 if k < 16 else K1_LAST
                nc.tensor.matmul(s1, lhsT=onesDn[0:rows, :],
                                 rhs=fr["zts"][k][0:rows, :],
                                 start=(k == 0), stop=(k == K1 - 1))
            s2_last = None
            for k in range(K1):
                rows = 128 if k < 16 else K1_LAST
                s2_last = nc.tensor.matmul(s2, lhsT=onesDp[0:rows, :],
                                           rhs=fr["sqs"][k][0:rows, :],
                                           start=(k == 0), stop=(k == K1 - 1))
            # aug row (-mu) for the L1 matmul.  s1/s2 share a PSUM bank, and
            # PE-write + ACT-read of the same bank is a fatal HW collision,
            # so the s1 readout must wait for the s2 group to finish.
            cp = nc.scalar.activation(fr["zt16"][K1_LAST:K1_LAST + 1, :], s1,
                                      AF.Copy)
            add_dep_helper(cp.ins, s2_last.ins, reason="psum bank collision")
            # L1 var ~= s2: E[z]^2 <= ~1e-2 * var even for 4.5-sigma rows,
            # so skip the mu^2 subtraction and rsqrt straight off the PSUM row.
            quake_rsqrt(s2, fr["invs1"][0:1, :], "q")
            fr["bc1"], fr["bc1_inst"] = bcast(fr["invs1"][0:1, :])

        def front_mm(it, fr, m):
            """One L1 matmul group + evac."""
            if m == 0:
                fr["h1"] = []
            py = ps_y.tile([128, NT], F32, tag="py")
            msl = slice(m * 128, (m + 1) * 128)
            for k in range(K1):
                nc.tensor.matmul(py[:, :], lhsT=w1[k][:, msl], rhs=fr["zts"][k],
                                 start=(k == 0), stop=(k == K1 - 1))
            ht = h_p.tile([128, NT], F16, tag=f"h1_{m}")
            evac(py, fr["bc1"], ht, cbT[:, m:m + 1])
            fr["h1"].append(ht)

        def back_stats(it, lidx, hcur):
            """LN stats for L2/L3: sums on PE, rsqrt on DVE."""
            sst = ps_s.tile([128, NT], F32, tag="sst", name="sst")
            s1 = sst[0:1, :]
            s2 = sst[32:33, :]
            for k in range(4):
                nc.tensor.matmul(s1, lhsT=onesn[:, :], rhs=hcur[k][:, :],
                                 start=(k == 0), stop=(k == 3))
            s2_last = None
            for k in range(4):
                sq = sqb_p.tile([128, NT], F16, tag="sq")
                if k % 2 == 0:
                    nc.vector.tensor_mul(sq[:, :], hcur[k][:, :], hcur[k][:, :])
                else:
                    nc.scalar.activation(sq[:, :], hcur[k][:, :], AF.Square)
                s2_last = nc.tensor.matmul(s2, lhsT=onesp[:, :], rhs=sq[:, :],
                                           start=(k == 0), stop=(k == 3))
            negmu = h_p.tile([1, NT], F16, tag=f"negmu_{lidx}")
            cp = nc.scalar.activation(negmu[:, :], s1, AF.Copy)
            add_dep_helper(cp.ins, s2_last.ins, reason="psum bank collision")
            vt = st_p.tile([1, NT], F32, tag="vt")
            mu2 = st_p.tile([1, NT], F32, tag="mu2")
            nc.vector.tensor_mul(mu2[:, :], negmu[:, :], negmu[:, :])
            nc.vector.tensor_sub(vt[:, :], s2, mu2[:, :])
            invs = st_p.tile([1, NT], F32, tag="invs", name="invs")
            quake_rsqrt(vt[0:1, :], invs[0:1, :], "q")
            bct, bct_inst = bcast(invs[0:1, :])
            return negmu, bct, bct_inst

        def back_main(it, lidx, hcur, negmu, bct, wts):
            hnew = []
            for m in range(4):
                py = ps_y.tile([128, NT], F32, tag="py")
                msl = slice(m * 128, (m + 1) * 128)
                for k in range(4):
                    nc.tensor.matmul(py[:, :], lhsT=wts[k][:, msl],
                                     rhs=hcur[k][:, :],
                                     start=(k == 0), stop=False)
                nc.tensor.matmul(py[:, :], lhsT=wts[4][:, msl], rhs=negmu[:, :],
                                 start=False, stop=True)
                ht = h_p.tile([128, NT], F16, tag=f"h_{lidx}_{m}")
                evac(py, bct, ht, cbT[:, lidx * 4 + m:lidx * 4 + m + 1])
                hnew.append(ht)
            return hnew

        def back_l4(it, hcur):
            bs = it * NT
            pq = ps_t.tile([1, NT], F32, tag="tpr")
            for k in range(4):
                nc.tensor.matmul(pq[:, :], lhsT=wo[:, k, :], rhs=hcur[k][:, :],
                                 start=(k == 0), stop=(k == 3))
            nc.scalar.activation(qrow[0:1, bs:bs + NT], pq[:, :], AF.Tanh,
                                 bias=boutT[:, :])

        def emit_front_all(it):
            fr = front_dma(it)
            front_sq(it, fr, range(K1))
            front_stat_mm(it, fr)
            for m in range(4):
                front_mm(it, fr, m)
            return fr

        # ---------- pipelined emission ----------
        pipelined = os.environ.get("KERNEL_PIPELINE", "1") == "1"
        if pipelined:
            fr = front_dma(0)
            front_sq(0, fr, range(K1))
            load_weights()
            front_stat_mm(0, fr)
            for m in range(4):
                front_mm(0, fr, m)
            for it in range(ntiles):
                nxt = None
                if it + 1 < ntiles:
                    nxt = front_dma(it + 1)
                    front_sq(it + 1, nxt, range(0, 9))
                st2 = back_stats(it, 1, fr["h1"])
                if nxt is not None:
                    front_sq(it + 1, nxt, range(9, K1))
                h2 = back_main(it, 1, fr["h1"], st2[0], st2[1], w2)
                if nxt is not None:
                    front_stat_mm(it + 1, nxt)
                st3 = back_stats(it, 2, h2)
                h3 = back_main(it, 2, h2, st3[0], st3[1], w3)
                if nxt is not None:
                    front_mm(it + 1, nxt, 0)
                    front_mm(it + 1, nxt, 1)
                back_l4(it, h3)
                if nxt is not None:
                    front_mm(it + 1, nxt, 2)
                    front_mm(it + 1, nxt, 3)
                    fr = nxt
        else:
            load_weights()
            for it in range(ntiles):
                fr = emit_front_all(it)
                st2 = back_stats(it, 1, fr["h1"])
                h2 = back_main(it, 1, fr["h1"], st2[0], st2[1], w2)
                st3 = back_stats(it, 2, h2)
                h3 = back_main(it, 2, h2, st3[0], st3[1], w3)
                back_l4(it, h3)

        nc.sync.dma_start(out=q_d[:, :], in_=qrow[:, :])


# ---------------- host side ----------------

def host_prep(x, a, g1, beta1, g2, beta2, g3, beta3,
              w1, b1, w2, b2, w3, b3, w_out, b_out):
    f16 = np.float16
    z = np.empty((x.shape[0], D), dtype=f16)
    np.multiply(x[:, :HALF], np.float32(1.0 / X_NORM), out=z[:, :HALF],
                casting="unsafe")
    np.multiply(x[:, HALF:], np.float32(1.0 / V_NORM), out=z[:, HALF:INPUT_DIM],
                casting="unsafe")
    z[:, INPUT_DIM:] = a.astype(f16)

    def fold(w, g, beta, b):
        wg = (w.astype(np.float64) * g.astype(np.float64)[None, :])
        rs = wg.sum(axis=1)
        c = w.astype(np.float64) @ beta.astype(np.float64) + b.astype(np.float64)
        out = np.empty((w.shape[1] + 1, w.shape[0]), dtype=f16)
        out[:w.shape[1]] = wg.T.astype(f16)
        out[w.shape[1]] = rs.astype(f16)
        return out, c.astype(np.float32)

    w1a, c1 = fold(w1, g1, beta1, b1)
    w2a, c2 = fold(w2, g2, beta2, b2)
    w3a, c3 = fold(w3, g3, beta3, b3)
    cb = np.empty((128, 12), np.float32)
    for li, c in enumerate((c1, c2, c3)):
        for m in range(4):
            cb[:, li * 4 + m] = c[m * 128:(m + 1) * 128]
    wout = w_out.T.astype(f16)  # [H, 1]
    bout = float(b_out[0])
    return z, w1a, w2a, w3a, cb, wout, bout


_NC_CACHE = {}


def make_in_maps(z, w1a, w2a, w3a, cb, wout):
    ntiles = BC // NT
    in_maps = []
    for c in range(NCORES):
        zc = z[c * BC:(c + 1) * BC]
        m = {"w1a": w1a, "w2a": w2a, "w3a": w3a, "cb": cb, "wout": wout}
        if ZT_CONTIG:
            # ztc[p, it*16*NT + k*NT + n] = z[it*NT + n, k*128 + p]
            zmain = zc[:, :INPUT_DIM].reshape(ntiles, NT, 16, 128)
            m["ztc"] = np.ascontiguousarray(
                zmain.transpose(3, 0, 2, 1).reshape(128, ntiles * 16 * NT))
            m["zt16r"] = np.ascontiguousarray(zc[:, INPUT_DIM:].T)
        else:
            m["zt"] = np.ascontiguousarray(zc.T)
        in_maps.append(m)
    return in_maps


def kernel(**inputs):
    inputs = {k: np.asarray(v) for k, v in inputs.items()}
    z, w1a, w2a, w3a, cb, wout, bout = host_prep(**inputs)

    key = (round(bout, 10), BC)
    if key not in _NC_CACHE:
        _NC_CACHE[key] = build_nc(bout, BC)
    nc = _NC_CACHE[key]

    in_maps = make_in_maps(z, w1a, w2a, w3a, cb, wout)
    res = run_bass_kernel_spmd(nc, in_maps, list(range(NCORES)))
    q = np.concatenate([res.results[c]["q"].reshape(BC, 1)
                        for c in range(NCORES)], axis=0).astype(np.float32)
    return q


def build_bench_nc(dmaq="scalar"):
    return build_nc(0.0, BC, dmaq=dmaq)


# revision 5
# speedup vs baseline: 1.0233x; 1.0233x over previous
"""Trainium2 Bass kernel for nn_Critic — v2: single-copy input (no zr).

Differences vs v1 baseline:
  - input is shipped ONCE (feature-major zt only); the batch-major copy
    (zr) and its 17MB/core of DMA are gone.
  - L1 LayerNorm stats come from the zt chunks themselves: s1 = -mu via
    (-1/D)-ones matmuls over the 17 chunks, s2 = E[z^2] via (1/D)-ones
    matmuls over squared chunks (squares split between DVE and ACT to
    balance engine load).  var = s2 - mu^2 (f-major), quake-rsqrt nr=1.
  - no PE transposes / identity matrix / batch-major quake needed at all.
  - optional contiguous host layout (ZT_CONTIG): z pre-reordered so each
    tile's 16 main chunks are ONE [128, 8192] contiguous DMA.
Everything else (folded LN weights, aug -mu row, invs evac on DVE, tanh
+bias on ACT, PE ones-matmul L2/L3 stats, software pipelining) as v1.
"""

import os
import sys
import numpy as np

for _p in ("/opt/trn_rl_repo",):
    if os.path.isdir(_p) and _p not in sys.path:
        sys.path.append(_p)

from contextlib import ExitStack

import concourse.bass as bass  # noqa: E402
import concourse.tile as tile  # noqa: E402
from concourse import bacc, mybir  # noqa: E402
from concourse.bass_utils import run_bass_kernel_spmd  # noqa: E402
from concourse.tile_rust import add_dep_helper  # noqa: E402

NCORES = 8
B = 32768
BC = B // NCORES
INPUT_DIM = 2048
HALF = INPUT_DIM // 2
N_ACTIONS = 32
D = INPUT_DIM + N_ACTIONS  # 2080
H = 512
NT = 512
EPS = 1e-5
X_NORM = 50.0
V_NORM = 10.0

F16 = mybir.dt.float16
F32 = mybir.dt.float32
I32 = mybir.dt.int32
AF = mybir.ActivationFunctionType
OP = mybir.AluOpType

K1 = 17
K1_LAST = D - 16 * 128  # 32
RSQRT_MAGIC = 0x5F3759DF

ZT_CONTIG = True


def build_nc(bout: float, bc: int = BC, dmaq: str = None):
    if dmaq is None:
        dmaq = "scalar4"
    ntiles = bc // NT
    assert ntiles * NT == bc

    nc = bacc.Bacc("TRN2", target_bir_lowering=False, debug=False,
                   num_devices=NCORES)

    if ZT_CONTIG:
        ztc_d = nc.dram_tensor("ztc", [128, ntiles * 16 * NT], F16,
                               kind="ExternalInput").ap()
        zt16_d = nc.dram_tensor("zt16r", [K1_LAST, bc], F16,
                                kind="ExternalInput").ap()
    else:
        ztc_d = nc.dram_tensor("zt", [D, bc], F16, kind="ExternalInput").ap()
        zt16_d = None
    w1_d = nc.dram_tensor("w1a", [D + 1, H], F16, kind="ExternalInput").ap()
    w2_d = nc.dram_tensor("w2a", [H + 1, H], F16, kind="ExternalInput").ap()
    w3_d = nc.dram_tensor("w3a", [H + 1, H], F16, kind="ExternalInput").ap()
    cb_d = nc.dram_tensor("cb", [128, 12], F32, kind="ExternalInput").ap()
    wo_d = nc.dram_tensor("wout", [H, 1], F16, kind="ExternalInput").ap()
    q_d = nc.dram_tensor("q", [1, bc], F32, kind="ExternalOutput").ap()

    with tile.TileContext(nc) as tc:
        _emit(tc, ntiles, bout, ztc_d, zt16_d, w1_d, w2_d, w3_d, cb_d, wo_d,
              q_d, dmaq)

    nc.compile()
    return nc


def _emit(tc, ntiles, bout, ztc_d, zt16_d, w1_d, w2_d, w3_d, cb_d, wo_d, q_d,
          dmaq="scalar"):
    nc = tc.nc
    with ExitStack() as ctx:
        wp = ctx.enter_context(tc.tile_pool(name="wp", bufs=1))
        zt_p = ctx.enter_context(tc.tile_pool(name="ztp", bufs=3))
        h_p = ctx.enter_context(tc.tile_pool(name="hp", bufs=2))
        u_p = ctx.enter_context(tc.tile_pool(name="up", bufs=4))
        sq_p = ctx.enter_context(tc.tile_pool(name="sqp", bufs=20))
        sqb_p = ctx.enter_context(tc.tile_pool(name="sqbp", bufs=4))
        bc_p = ctx.enter_context(tc.tile_pool(name="bcp", bufs=3))
        st_p = ctx.enter_context(tc.tile_pool(name="stp", bufs=3))
        ps_y = ctx.enter_context(tc.tile_pool(name="psy", bufs=4, space="PSUM"))
        # one bank per stats pair: s1 at partition row 0, s2 at row 32
        ps_s = ctx.enter_context(tc.tile_pool(name="pss", bufs=3, space="PSUM"))
        ps_t = ctx.enter_context(tc.tile_pool(name="pst", bufs=1, space="PSUM"))

        # ---- persistent constants / weights ----
        w1main = wp.tile([128, 16, H], F16, tag="w1main")
        w1last = wp.tile([K1_LAST + 1, H], F16, tag="w1last")
        w1 = [w1main[:, k, :] for k in range(16)] + [w1last]
        w2main = wp.tile([128, 4, H], F16, tag="w2main")
        w2aug = wp.tile([1, H], F16, tag="w2aug")
        w2 = [w2main[:, k, :] for k in range(4)] + [w2aug]
        w3main = wp.tile([128, 4, H], F16, tag="w3main")
        w3aug = wp.tile([1, H], F16, tag="w3aug")
        w3 = [w3main[:, k, :] for k in range(4)] + [w3aug]
        cbT = wp.tile([128, 12], F32, tag="cbT")
        wo = wp.tile([128, 4, 1], F16, tag="wo")

        def load_weights():
            for hlf in range(2):
                nc.sync.dma_start(
                    out=w1main[:, hlf * 8:(hlf + 1) * 8, :],
                    in_=w1_d[hlf * 1024:(hlf + 1) * 1024, :]
                        .rearrange("(k p) h -> p k h", k=8))
            nc.sync.dma_start(out=w1last[:, :],
                              in_=w1_d[2048:2048 + K1_LAST + 1, :])
            wq = nc.sync if dmaq in ("scalar", "scalar4") else nc.scalar
            for wt, wa, wd in ((w2main, w2aug, w2_d), (w3main, w3aug, w3_d)):
                wq.dma_start(
                    out=wt[:, :, :],
                    in_=wd[0:H, :].rearrange("(k p) h -> p k h", k=4))
                wq.dma_start(out=wa[:, :], in_=wd[H:H + 1, :])
            wq.dma_start(out=cbT[:, :], in_=cb_d[:, :])
            wq.dma_start(
                out=wo[:, :, :],
                in_=wo_d[:, :].rearrange("(k p) o -> p k o", k=4))

        onesn = wp.tile([128, 1], F16, tag="onesn")  # -1/H (L2/L3 s1)
        nc.vector.memset(onesn[:, :], -1.0 / H)
        onesp = wp.tile([128, 1], F16, tag="onesp")  # +1/H (L2/L3 s2)
        nc.vector.memset(onesp[:, :], 1.0 / H)
        onesDn = wp.tile([128, 1], F16, tag="onesDn")  # -1/D (L1 s1)
        nc.vector.memset(onesDn[:, :], -1.0 / D)
        onesDp = wp.tile([128, 1], F16, tag="onesDp")  # +1/D (L1 s2)
        nc.vector.memset(onesDp[:, :], 1.0 / D)
        boutT = wp.tile([1, 1], F32, tag="boutT")
        nc.vector.memset(boutT[:, :], bout)
        qrow = wp.tile([1, ntiles * NT], F32, tag="qrow")

        def evac(py, bctile, htile, cbcol):
            """h = tanh(invs (.) psum + c).

            Split into [PSUM->SBUF f16 copy] + [f16 x f16 multiply] so the
            PSUM bank is released as soon as the matmul group finishes --
            the multiply (which waits on the invs broadcast, i.e. the whole
            stats/rsqrt chain) no longer gates PE's psum rotation."""
            u1 = u_p.tile([128, NT], F16, tag="u1")
            nc.vector.tensor_copy(u1[:, :], py[:, :])
            u = u_p.tile([128, NT], F16, tag="u")
            nc.vector.tensor_mul(u[:, :], u1[:, :], bctile[:, :])
            nc.scalar.activation(htile[:, :], u[:, :], AF.Tanh, bias=cbcol)

        def bcast(row_ap):
            t = bc_p.tile([128, NT], F16, tag="bc")
            inst = nc.gpsimd.partition_broadcast(t[:, :], row_ap)
            return t, inst

        def quake_rsqrt(v_ap, out_ap, tag, rows=1, nr=1):
            """out = 1/sqrt(v) elementwise; v, out: [rows, w] f32 SBUF."""
            w = v_ap.shape[-1]
            it = st_p.tile([rows, w], I32, tag=f"qi_{tag}", name=f"qi_{tag}")
            tt = st_p.tile([rows, w], F32, tag=f"qt_{tag}", name=f"qt_{tag}")
            nc.vector.tensor_scalar(out=it[:, :], in0=v_ap.bitcast(I32),
                                    scalar1=1, scalar2=None,
                                    op0=OP.arith_shift_right)
            nc.vector.tensor_scalar(out=it[:, :], in0=it[:, :],
                                    scalar1=-1, scalar2=RSQRT_MAGIC,
                                    op0=OP.mult, op1=OP.add)
            y = it[:, :].bitcast(F32)
            for r in range(nr):
                nc.vector.tensor_mul(tt[:, :], y, y)
                nc.vector.tensor_mul(tt[:, :], tt[:, :], v_ap)
                nc.vector.tensor_scalar(out=tt[:, :], in0=tt[:, :],
                                        scalar1=-0.5, scalar2=1.5,
                                        op0=OP.mult, op1=OP.add)
                nc.vector.tensor_mul(out_ap, y, tt[:, :])
                y = out_ap

        # ---------- per-tile emission pieces ----------

        def front_dma(it):
            """Allocate tile state + start input DMAs."""
            bs = it * NT
            fr = {"bs": bs}
            fr["zt16"] = zt_p.tile([K1_LAST + 1, NT], F16, tag="zt16",
                                   name="zt16")
            fr["invs1"] = st_p.tile([1, NT], F16, tag="invs", name="invs")
            ztmain = zt_p.tile([128, 16, NT], F16, tag="ztmain")
            if ZT_CONTIG:
                if dmaq == "split":
                    nc.sync.dma_start(
                        out=ztmain[:, 0:8, :],
                        in_=ztc_d[:, it * 16 * NT:it * 16 * NT + 8 * NT]
                            .rearrange("p (k n) -> p k n", k=8))
                    nc.scalar.dma_start(
                        out=ztmain[:, 8:16, :],
                        in_=ztc_d[:, it * 16 * NT + 8 * NT:(it + 1) * 16 * NT]
                            .rearrange("p (k n) -> p k n", k=8))
                elif dmaq == "scalar4":
                    for pc in range(4):
                        nc.scalar.dma_start(
                            out=ztmain[:, pc * 4:(pc + 1) * 4, :],
                            in_=ztc_d[:, it * 16 * NT + pc * 4 * NT:
                                      it * 16 * NT + (pc + 1) * 4 * NT]
                                .rearrange("p (k n) -> p k n", k=4))
                else:
                    q = nc.sync if dmaq == "sync" else nc.scalar
                    q.dma_start(
                        out=ztmain[:, :, :],
                        in_=ztc_d[:, it * 16 * NT:(it + 1) * 16 * NT]
                            .rearrange("p (k n) -> p k n", k=16))
                nc.sync.dma_start(out=fr["zt16"][0:K1_LAST, :],
                                  in_=zt16_d[:, bs:bs + NT])
            else:
                for hlf in range(2):
                    nc.scalar.dma_start(
                        out=ztmain[:, hlf * 8:(hlf + 1) * 8, :],
                        in_=ztc_d[hlf * 1024:(hlf + 1) * 1024, bs:bs + NT]
                            .rearrange("(k p) n -> p k n", k=8))
                nc.sync.dma_start(out=fr["zt16"][0:K1_LAST, :],
                                  in_=ztc_d[2048:2048 + K1_LAST, bs:bs + NT])
            fr["zts"] = [ztmain[:, k, :] for k in range(16)] + [fr["zt16"]]
            return fr

        def front_sq(it, fr, ks):
            """Square chunks ks; alternate DVE/ACT to balance load."""
            if "sqs" not in fr:
                fr["sqs"] = {}
            for k in ks:
                src = fr["zts"][k]
                rows = 128 if k < 16 else K1_LAST
                sq = sq_p.tile([128, NT], F16, tag="sq1", name="sq1")
                if k % 2 == 0:
                    nc.vector.tensor_mul(sq[0:rows, :], src[0:rows, :],
                                         src[0:rows, :])
                else:
                    nc.scalar.activation(sq[0:rows, :], src[0:rows, :],
                                         AF.Square)
                fr["sqs"][k] = sq

        def front_stat_mm(it, fr):
            """s1 = -mu, s2 = E[z^2] on PE; then var, rsqrt, aug row."""
            sst = ps_s.tile([128, NT], F32, tag="sst", name="sst")
            s1 = sst[0:1, :]
            s2 = sst[32:33, :]
            for k in range(K1):
                rows = 128 if k < 16 else K1_LAST
                nc.tensor.matmul(s1, lhsT=onesDn[0:rows, :],
                                 rhs=fr["zts"][k][0:rows, :],
                                 start=(k == 0), stop=(k == K1 - 1))
            s2_last = None
            for k in range(K1):
                rows = 128 if k < 16 else K1_LAST
                s2_last = nc.tensor.matmul(s2, lhsT=onesDp[0:rows, :],
                                           rhs=fr["sqs"][k][0:rows, :],
                                           start=(k == 0), stop=(k == K1 - 1))
            # aug row (-mu) for the L1 matmul.  s1/s2 share a PSUM bank, and
            # PE-write + ACT-read of the same bank is a fatal HW collision,
            # so the s1 readout must wait for the s2 group to finish.
            cp = nc.scalar.activation(fr["zt16"][K1_LAST:K1_LAST + 1, :], s1,
                                      AF.Copy)
            add_dep_helper(cp.ins, s2_last.ins, reason="psum bank collision")
            # L1 var ~= s2: E[z]^2 <= ~1e-2 * var even for 4.5-sigma rows,
            # so skip the mu^2 subtraction and rsqrt straight off the PSUM row.
            quake_rsqrt(s2, fr["invs1"][0:1, :], "q")
            fr["bc1"], fr["bc1_inst"] = bcast(fr["invs1"][0:1, :])

        def front_mm(it, fr, m):
            """One L1 matmul group + evac."""
            if m == 0:
                fr["h1"] = []
            py = ps_y.tile([128, NT], F32, tag="py")
            msl = slice(m * 128, (m + 1) * 128)
            for k in range(K1):
                nc.tensor.matmul(py[:, :], lhsT=w1[k][:, msl], rhs=fr["zts"][k],
                                 start=(k == 0), stop=(k == K1 - 1))
            ht = h_p.tile([128, NT], F16, tag=f"h1_{m}")
            evac(py, fr["bc1"], ht, cbT[:, m:m + 1])
            fr["h1"].append(ht)

        def back_stats(it, lidx, hcur):
            """LN stats for L2/L3: sums on PE, rsqrt on DVE."""
            sst = ps_s.tile([128, NT], F32, tag="sst", name="sst")
            s1 = sst[0:1, :]
            s2 = sst[32:33, :]
            for k in range(4):
                nc.tensor.matmul(s1, lhsT=onesn[:, :], rhs=hcur[k][:, :],
                                 start=(k == 0), stop=(k == 3))
            s2_last = None
            for k in range(4):
                sq = sqb_p.tile([128, NT], F16, tag="sq")
                if k % 2 == 0:
                    nc.vector.tensor_mul(sq[:, :], hcur[k][:, :], hcur[k][:, :])
                else:
                    nc.scalar.activation(sq[:, :], hcur[k][:, :], AF.Square)
                s2_last = nc.tensor.matmul(s2, lhsT=onesp[:, :], rhs=sq[:, :],
                                           start=(k == 0), stop=(k == 3))
            negmu = h_p.tile([1, NT], F16, tag=f"negmu_{lidx}")
            cp = nc.scalar.activation(negmu[:, :], s1, AF.Copy)
            add_dep_helper(cp.ins, s2_last.ins, reason="psum bank collision")
            vt = st_p.tile([1, NT], F32, tag="vt")
            mu2 = st_p.tile([1, NT], F32, tag="mu2")
            nc.vector.tensor_mul(mu2[:, :], negmu[:, :], negmu[:, :])
            nc.vector.tensor_sub(vt[:, :], s2, mu2[:, :])
            invs = st_p.tile([1, NT], F16, tag="invs", name="invs")
            quake_rsqrt(vt[0:1, :], invs[0:1, :], "q")
            bct, bct_inst = bcast(invs[0:1, :])
            return negmu, bct, bct_inst

        def back_main(it, lidx, hcur, negmu, bct, wts):
            hnew = []
            for m in range(4):
                py = ps_y.tile([128, NT], F32, tag="py")
                msl = slice(m * 128, (m + 1) * 128)
                for k in range(4):
                    nc.tensor.matmul(py[:, :], lhsT=wts[k][:, msl],
                                     rhs=hcur[k][:, :],
                                     start=(k == 0), stop=False)
                nc.tensor.matmul(py[:, :], lhsT=wts[4][:, msl], rhs=negmu[:, :],
                                 start=False, stop=True)
                ht = h_p.tile([128, NT], F16, tag=f"h_{lidx}_{m}")
                evac(py, bct, ht, cbT[:, lidx * 4 + m:lidx * 4 + m + 1])
                hnew.append(ht)
            return hnew

        def back_l4(it, hcur):
            bs = it * NT
            pq = ps_t.tile([1, NT], F32, tag="tpr")
            for k in range(4):
                nc.tensor.matmul(pq[:, :], lhsT=wo[:, k, :], rhs=hcur[k][:, :],
                                 start=(k == 0), stop=(k == 3))
            nc.scalar.activation(qrow[0:1, bs:bs + NT], pq[:, :], AF.Tanh,
                                 bias=boutT[:, :])

        def emit_front_all(it):
            fr = front_dma(it)
            front_sq(it, fr, range(K1))
            front_stat_mm(it, fr)
            for m in range(4):
                front_mm(it, fr, m)
            return fr

        # ---------- pipelined emission ----------
        pipelined = os.environ.get("KERNEL_PIPELINE", "1") == "1"
        if pipelined:
            fr = front_dma(0)
            front_sq(0, fr, range(K1))
            load_weights()
            front_stat_mm(0, fr)
            for m in range(4):
                front_mm(0, fr, m)
            for it in range(ntiles):
                nxt = None
                if it + 1 < ntiles:
                    nxt = front_dma(it + 1)
                    front_sq(it + 1, nxt, range(0, 9))
                st2 = back_stats(it, 1, fr["h1"])
                if nxt is not None:
                    front_sq(it + 1, nxt, range(9, K1))
                h2 = back_main(it, 1, fr["h1"], st2[0], st2[1], w2)
                if nxt is not None:
                    front_stat_mm(it + 1, nxt)
                    front_mm(it + 1, nxt, 0)
                    front_mm(it + 1, nxt, 1)
                st3 = back_stats(it, 2, h2)
                if nxt is not None:
                    front_mm(it + 1, nxt, 2)
                h3 = back_main(it, 2, h2, st3[0], st3[1], w3)
                if nxt is not None:
                    front_mm(it + 1, nxt, 3)
                back_l4(it, h3)
                if nxt is not None:
                    fr = nxt
        else:
            load_weights()
            for it in range(ntiles):
                fr = emit_front_all(it)
                st2 = back_stats(it, 1, fr["h1"])
                h2 = back_main(it, 1, fr["h1"], st2[0], st2[1], w2)
                st3 = back_stats(it, 2, h2)
                h3 = back_main(it, 2, h2, st3[0], st3[1], w3)
                back_l4(it, h3)

        nc.sync.dma_start(out=q_d[:, :], in_=qrow[:, :])


# ---------------- host side ----------------

def host_prep(x, a, g1, beta1, g2, beta2, g3, beta3,
              w1, b1, w2, b2, w3, b3, w_out, b_out):
    f16 = np.float16
    z = np.empty((x.shape[0], D), dtype=f16)
    np.multiply(x[:, :HALF], np.float32(1.0 / X_NORM), out=z[:, :HALF],
                casting="unsafe")
    np.multiply(x[:, HALF:], np.float32(1.0 / V_NORM), out=z[:, HALF:INPUT_DIM],
                casting="unsafe")
    z[:, INPUT_DIM:] = a.astype(f16)

    def fold(w, g, beta, b):
        wg = (w.astype(np.float64) * g.astype(np.float64)[None, :])
        rs = wg.sum(axis=1)
        c = w.astype(np.float64) @ beta.astype(np.float64) + b.astype(np.float64)
        out = np.empty((w.shape[1] + 1, w.shape[0]), dtype=f16)
        out[:w.shape[1]] = wg.T.astype(f16)
        out[w.shape[1]] = rs.astype(f16)
        return out, c.astype(np.float32)

    w1a, c1 = fold(w1, g1, beta1, b1)
    w2a, c2 = fold(w2, g2, beta2, b2)
    w3a, c3 = fold(w3, g3, beta3, b3)
    cb = np.empty((128, 12), np.float32)
    for li, c in enumerate((c1, c2, c3)):
        for m in range(4):
            cb[:, li * 4 + m] = c[m * 128:(m + 1) * 128]
    wout = w_out.T.astype(f16)  # [H, 1]
    bout = float(b_out[0])
    return z, w1a, w2a, w3a, cb, wout, bout


_NC_CACHE = {}


def make_in_maps(z, w1a, w2a, w3a, cb, wout):
    ntiles = BC // NT
    in_maps = []
    for c in range(NCORES):
        zc = z[c * BC:(c + 1) * BC]
        m = {"w1a": w1a, "w2a": w2a, "w3a": w3a, "cb": cb, "wout": wout}
        if ZT_CONTIG:
            # ztc[p, it*16*NT + k*NT + n] = z[it*NT + n, k*128 + p]
            zmain = zc[:, :INPUT_DIM].reshape(ntiles, NT, 16, 128)
            m["ztc"] = np.ascontiguousarray(
                zmain.transpose(3, 0, 2, 1).reshape(128, ntiles * 16 * NT))
            m["zt16r"] = np.ascontiguousarray(zc[:, INPUT_DIM:].T)
        else:
            m["zt"] = np.ascontiguousarray(zc.T)
        in_maps.append(m)
    return in_maps


def kernel(**inputs):
    inputs = {k: np.asarray(v) for k, v in inputs.items()}
    z, w1a, w2a, w3a, cb, wout, bout = host_prep(**inputs)

    key = (round(bout, 10), BC)
    if key not in _NC_CACHE:
        _NC_CACHE[key] = build_nc(bout, BC)
    nc = _NC_CACHE[key]

    in_maps = make_in_maps(z, w1a, w2a, w3a, cb, wout)
    res = run_bass_kernel_spmd(nc, in_maps, list(range(NCORES)))
    q = np.concatenate([res.results[c]["q"].reshape(BC, 1)
                        for c in range(NCORES)], axis=0).astype(np.float32)
    return q


def build_bench_nc(dmaq="scalar"):
    return build_nc(0.0, BC, dmaq=dmaq)
